# revision 17
# baseline (speedup 1.0000x reference)
"""Distributed multi-head attention (RoPE, non-causal) on 8 TRN2 NeuronCores.

Sharding: tensor-parallel over heads. Core c owns heads {2c, 2c+1}:
  - wq/wk/wv rows c*256:(c+1)*256 (output dim), x replicated (pre-transposed),
  - attention computed locally per (batch, head),
  - per-batch AllGather of the attention outputs (transposed layout, bf16),
  - each core then computes output columns c*256:(c+1)*256 with its wo rows.
Host side only shards/casts inputs and concatenates the 8 output column
slices -- all FLOPs run on device.

Structure: software-pipelined per batch so the ScalarE exp stream, the
AllGather, and the wo stage all overlap TensorE work of neighboring
batches:  proj(b) -> attn(b) -> AG(b) -> [wo(b-1)] ...

Layout/precision tricks:
  - All matmuls in bf16 (PSUM accumulates f32); rel-err ~5e-3.
  - RoPE pairs (even/odd head-dim) are separated into halves by permuting the
    wq/wk ROWS on the host, so on device RoPE is 4 full-width DVE ops against
    duplicated cos/sin tables. Permutation is applied identically to q and k,
    so q.k is unchanged.
  - Scores are computed transposed ([k, q]) so exp(scores) feeds attn@v as
    lhsT with no transpose; softmax denominator comes from a ones-column
    appended to v (matmul N=129). No max-subtraction: |scores| < ~10 here.
  - attn output normalized per-partition (q) then PE-transposed to [hd, q]
    so the AllGather concatenates cores along the o dim.
  - wo stage computes out.T (N=512 moving dim); host transposes back.
"""

import numpy as np
import ml_dtypes

B, S, D, H = 4, 2048, 2048, 16
HD = 128            # head dim
NCORES = 8
HPC = H // NCORES   # heads per core = 2
OSL = HPC * HD      # per-core o-slice = 256
ROWS = B * S        # 8192 flattened rows
DCH = D // 128      # 16 contraction chunks
SCH = 512           # seq chunk for projections
KB = S // 128       # 16 k-blocks per batch
QC = 512            # q chunk in attention
NQC = S // QC       # 4

BF16 = ml_dtypes.bfloat16
_NC_CACHE = None


def _build():
    import concourse.bass as bass  # noqa: F401
    import concourse.mybir as mybir
    import concourse.tile as tile
    from concourse import bacc
    from concourse.masks import make_identity

    fp32 = mybir.dt.float32
    bf16 = mybir.dt.bfloat16

    nc = bacc.Bacc(
        "TRN2",
        target_bir_lowering=False,
        debug=False,
        num_devices=NCORES,
    )

    xT = nc.declare_dram_parameter("xT", [D, ROWS], bf16, isOutput=False)
    wqT = nc.declare_dram_parameter("wqT", [D, OSL], bf16, isOutput=False)
    wkT = nc.declare_dram_parameter("wkT", [D, OSL], bf16, isOutput=False)
    wvT = nc.declare_dram_parameter("wvT", [D, OSL], bf16, isOutput=False)
    woT = nc.declare_dram_parameter("woT", [D, OSL], bf16, isOutput=False)
    cosd = nc.declare_dram_parameter("cosd", [128, S], fp32, isOutput=False)
    sind = nc.declare_dram_parameter("sind", [128, S], fp32, isOutput=False)
    outp = nc.declare_dram_parameter("out", [OSL, ROWS], fp32, isOutput=True)

    inv_sqrt_hd = 1.0 / float(np.sqrt(HD))

    with tile.TileContext(nc) as tc:
        with (
            tc.tile_pool(name="glob", bufs=1) as glob,
            tc.tile_pool(name="dram", bufs=1, space="DRAM") as dram,
            tc.tile_pool(name="qkv", bufs=2) as qkv,
            tc.tile_pool(name="xtp", bufs=3) as xtp,
            tc.tile_pool(name="attp", bufs=2) as attp,
            tc.tile_pool(name="gtp", bufs=3) as gtp,
            tc.tile_pool(name="tmpp", bufs=3) as tmpp,
            tc.tile_pool(name="smalls", bufs=4) as smalls,
            tc.tile_pool(name="otp", bufs=2) as otp,
            tc.tile_pool(name="psA", bufs=3, space="PSUM") as psA,
            tc.tile_pool(name="psB", bufs=3, space="PSUM") as psB,
            tc.tile_pool(name="psCD", bufs=2, space="PSUM") as psCD,
        ):
            ident = glob.tile([128, 128], bf16, name="ident")
            make_identity(nc, ident[:])

            wq_sb = glob.tile([128, DCH, OSL], bf16, name="wq_sb")
            wk_sb = glob.tile([128, DCH, OSL], bf16, name="wk_sb")
            wv_sb = glob.tile([128, DCH, OSL], bf16, name="wv_sb")
            wo_sb = glob.tile([128, DCH, OSL], bf16, name="wo_sb")
            cosb = glob.tile([128, S], fp32, name="cosb")
            sinb = glob.tile([128, S], fp32, name="sinb")
            # startup order: what the first projection needs, first
            nc.gpsimd.dma_start(
                wq_sb[:], wqT[:, :].rearrange("(c p) n -> p c n", p=128))
            xh00 = []
            for half in range(2):
                xth = xtp.tile([128, DCH // 2, SCH], bf16,
                               name=f"xt0_{half}", tag="xt")
                nc.gpsimd.dma_start(
                    xth[:],
                    xT[half * 1024:(half + 1) * 1024, 0:SCH]
                    .rearrange("(c p) n -> p c n", p=128))
                xh00.append(xth)
            nc.gpsimd.dma_start(cosb[:], cosd[:, :])
            nc.gpsimd.dma_start(sinb[:], sind[:, :])
            nc.gpsimd.dma_start(
                wk_sb[:], wkT[:, :].rearrange("(c p) n -> p c n", p=128))
            nc.gpsimd.dma_start(
                wv_sb[:], wvT[:, :].rearrange("(c p) n -> p c n", p=128))

            # half-batch granularity: cols [0:1024) and [1024:2048) of each
            # batch gather/project independently to shrink the serial tail
            HB = S // 2
            bounce = [dram.tile([OSL, HB], bf16, name=f"bounce{u}")
                      for u in range(2 * B)]
            gath = [dram.tile([NCORES * OSL, HB], bf16, addr_space="Shared",
                              name=f"gath{u}") for u in range(2 * B)]

            def fetch_x(b, sc):
                col0 = b * S + sc * SCH
                xh = []
                for half in range(2):
                    xth = xtp.tile([128, DCH // 2, SCH], bf16,
                                   name=f"xt{half}", tag="xt")
                    nc.gpsimd.dma_start(
                        xth[:],
                        xT[half * 1024:(half + 1) * 1024, col0:col0 + SCH]
                        .rearrange("(c p) n -> p c n", p=128))
                    xh.append(xth)
                return xh

            def proj(b, xh_pre):
                qt = qkv.tile([128, HPC, S], bf16, name="qt", tag="qt")
                kt = qkv.tile([128, HPC, S], bf16, name="kt", tag="kt")
                vt = qkv.tile([128, KB, HPC, HD + 1], bf16, name="vt",
                              tag="vt")
                nc.vector.memset(vt[:, :, :, HD:HD + 1], 1.0)
                for sc in range(S // SCH):
                    xh = xh_pre if sc == 0 else fetch_x(b, sc)
                    cosr = cosb[:, sc * SCH:(sc + 1) * SCH]
                    sinr = sinb[:, sc * SCH:(sc + 1) * SCH]
                    for (w_sb, dstT) in ((wq_sb, qt), (wk_sb, kt)):
                        for h in range(HPC):
                            ps = psA.tile([128, SCH], fp32, name="ps_proj",
                                          tag="psA")
                            for c in range(DCH):
                                nc.tensor.matmul(
                                    ps[:],
                                    w_sb[:, c, h * HD:(h + 1) * HD],
                                    xh[c // 8][:, c % 8, :],
                                    start=(c == 0), stop=(c == DCH - 1))
                            m1 = tmpp.tile([128, SCH], fp32, name="m1",
                                           tag="t")
                            m2 = tmpp.tile([128, SCH], fp32, name="m2",
                                           tag="t")
                            # m1 = [tr*cos ; ti*cos]; m2 swapped-halves =
                            # [ti*sin ; tr*sin] so later DVE ops use equal
                            # SBUF base partitions (PSUM operand may differ).
                            nc.vector.tensor_mul(m1[:], ps[:], cosr)
                            nc.vector.tensor_mul(
                                m2[0:64, :], ps[64:128, :], sinr[0:64, :])
                            nc.vector.tensor_mul(
                                m2[64:128, :], ps[0:64, :], sinr[64:128, :])
                            sl = slice(sc * SCH, (sc + 1) * SCH)
                            nc.vector.tensor_sub(
                                dstT[0:64, h, sl], m1[0:64, :], m2[0:64, :])
                            nc.vector.tensor_add(
                                dstT[64:128, h, sl], m2[64:128, :],
                                m1[64:128, :])
                    for ssb in range(SCH // 128):
                        kb = sc * (SCH // 128) + ssb
                        psv = psA.tile([128, OSL], fp32, name="psv", tag="psA")
                        for c in range(DCH):
                            nc.tensor.matmul(
                                psv[:],
                                xh[c // 8][:, c % 8, ssb * 128:(ssb + 1) * 128],
                                wv_sb[:, c, :],
                                start=(c == 0), stop=(c == DCH - 1))
                        nc.vector.tensor_copy(
                            vt[:, kb, :, 0:HD],
                            psv[:].rearrange("p (h d) -> p h d", h=HPC))
                return qt, kt, vt

            def attn_scores(qt, kt, h, qc):
                expT = attp.tile([128, KB, QC], bf16, name="expT", tag="expT")
                for kb in range(KB):
                    pss = psB.tile([128, QC], fp32, name="pss", tag="psB")
                    nc.tensor.matmul(
                        pss[:],
                        kt[:, h, kb * 128:(kb + 1) * 128],
                        qt[:, h, qc * QC:(qc + 1) * QC],
                        start=True, stop=True)
                    nc.scalar.activation(
                        expT[:, kb, :], pss[:],
                        mybir.ActivationFunctionType.Exp,
                        scale=inv_sqrt_hd)
                return expT

            def attn_v(vt, expT, b, h, qc):
                u = 2 * b + qc // 2
                for qsb in range(QC // 128):
                    pso = psCD.tile([128, HD + 1], fp32, name="pso",
                                    tag="psCD")
                    for kb in range(KB):
                        nc.tensor.matmul(
                            pso[:],
                            expT[:, kb, qsb * 128:(qsb + 1) * 128],
                            vt[:, kb, h, :],
                            start=(kb == 0), stop=(kb == KB - 1))
                    rc = smalls.tile([128, 1], fp32, name="rc", tag="rc")
                    nc.vector.reciprocal(rc[:], pso[:, HD:HD + 1])
                    a_sb = smalls.tile([128, HD], bf16, name="a_sb",
                                       tag="a_sb")
                    nc.vector.tensor_scalar_mul(a_sb[:], pso[:, 0:HD], rc[:])
                    pst = psCD.tile([128, 128], bf16, name="pst", tag="psCD")
                    nc.tensor.transpose(pst[:], a_sb[:], ident[:])
                    a_t = smalls.tile([128, 128], bf16, name="a_t", tag="a_t")
                    nc.vector.tensor_copy(a_t[:], pst[:])
                    col0 = (qc % 2) * QC + qsb * 128
                    nc.gpsimd.dma_start(
                        bounce[u][h * HD:(h + 1) * HD, col0:col0 + 128],
                        a_t[:])

            def attention(b, qt, kt, vt, post_half):
                units = [(h, qc) for qc in range(NQC) for h in range(HPC)]
                pend = []
                done = [0]

                def flush_one():
                    eT, ph, pqc = pend.pop(0)
                    attn_v(vt, eT, b, ph, pqc)
                    done[0] += 1
                    if done[0] == 4:
                        post_half(0)
                    elif done[0] == 8:
                        post_half(1)

                for (h, qc) in units:
                    expT = attn_scores(qt, kt, h, qc)
                    pend.append((expT, h, qc))
                    if len(pend) > 1:
                        flush_one()
                flush_one()

            def allgather(u):
                nc.gpsimd.collective_compute(
                    "AllGather",
                    mybir.AluOpType.bypass,
                    ins=[bounce[u].opt()],
                    outs=[gath[u].opt()],
                    replica_groups=[list(range(NCORES))],
                )

            def wo_stage(b, half):
                u = 2 * b + half
                for rc_ in range(2):
                    gh = []
                    for dh in range(2):
                        g = gtp.tile([128, DCH // 2, 512], bf16,
                                     name=f"gt{dh}", tag="gt")
                        nc.scalar.dma_start(
                            g[:],
                            gath[u][dh * 1024:(dh + 1) * 1024,
                                    rc_ * 512:(rc_ + 1) * 512]
                            .rearrange("(c p) n -> p c n", p=128))
                        gh.append(g)
                    for oc in range(OSL // 128):
                        psw = psA.tile([128, 512], fp32, name="psw", tag="psA")
                        for c in range(DCH):
                            nc.tensor.matmul(
                                psw[:],
                                wo_sb[:, c, oc * 128:(oc + 1) * 128],
                                gh[c // 8][:, c % 8, :],
                                start=(c == 0), stop=(c == DCH - 1))
                        out_t = otp.tile([128, 512], fp32, name="out_t",
                                         tag="out_t")
                        nc.vector.tensor_copy(out_t[:], psw[:])
                        col0 = b * S + half * HB + rc_ * 512
                        nc.scalar.dma_start(
                            outp[oc * 128:(oc + 1) * 128, col0:col0 + 512],
                            out_t[:])

            wo_loaded = [False]

            def make_post(b):
                def cb(half):
                    allgather(2 * b + half)
                    if not wo_loaded[0]:
                        nc.gpsimd.dma_start(
                            wo_sb[:],
                            woT[:, :].rearrange("(c p) n -> p c n", p=128))
                        wo_loaded[0] = True
                    if b >= 1:
                        wo_stage(b - 1, half)
                return cb

            xh_pre = xh00
            for b in range(B):
                qt, kt, vt = proj(b, xh_pre)
                if b + 1 < B:
                    xh_pre = fetch_x(b + 1, 0)
                attention(b, qt, kt, vt, make_post(b))
            wo_stage(B - 1, 0)
            wo_stage(B - 1, 1)

    nc.compile()
    return nc


def _shard_inputs(x, freqs_cos, freqs_sin, wq, wk, wv, wo):
    xf = np.asarray(x, dtype=np.float32).reshape(ROWS, D)
    xT = np.ascontiguousarray(xf.T).astype(BF16)
    fcT = np.asarray(freqs_cos, dtype=np.float32).T  # [64, S]
    fsT = np.asarray(freqs_sin, dtype=np.float32).T
    cosd = np.ascontiguousarray(np.concatenate([fcT, fcT], 0))  # [128, S]
    sind = np.ascontiguousarray(np.concatenate([fsT, fsT], 0))
    # even indices (real half) then odd (imag half), per head
    perm = np.concatenate([np.arange(0, HD, 2), np.arange(1, HD, 2)])
    in_maps = []
    for c in range(NCORES):
        rows = slice(c * OSL, (c + 1) * OSL)
        wq_c = np.asarray(wq)[rows].reshape(HPC, HD, D)[:, perm, :].reshape(OSL, D)
        wk_c = np.asarray(wk)[rows].reshape(HPC, HD, D)[:, perm, :].reshape(OSL, D)
        in_maps.append({
            "xT": xT,
            "wqT": np.ascontiguousarray(wq_c.T).astype(BF16),
            "wkT": np.ascontiguousarray(wk_c.T).astype(BF16),
            "wvT": np.ascontiguousarray(np.asarray(wv)[rows].T).astype(BF16),
            "woT": np.ascontiguousarray(np.asarray(wo)[rows].T).astype(BF16),
            "cosd": cosd,
            "sind": sind,
        })
    return in_maps


def run(inputs, trace=False, trace_cores=None):
    """Build (cached), run on 8 cores; returns (full_output, BassKernelResults)."""
    global _NC_CACHE
    from concourse.bass_utils import run_bass_kernel_spmd
    if _NC_CACHE is None:
        _NC_CACHE = _build()
    in_maps = _shard_inputs(**inputs)
    res = run_bass_kernel_spmd(
        _NC_CACHE, in_maps, core_ids=list(range(NCORES)), trace=trace,
        trace_cores=trace_cores)
    parts = [np.ascontiguousarray(
        np.asarray(res.results[c]["out"], dtype=np.float32).T)
        for c in range(NCORES)]
    full = np.concatenate(parts, axis=1).reshape(B, S, D)
    return full, res


def kernel(x, freqs_cos, freqs_sin, wq, wk, wv, wo):
    full, _ = run(dict(x=x, freqs_cos=freqs_cos, freqs_sin=freqs_sin,
                       wq=wq, wk=wk, wv=wv, wo=wo))
    return full


# revision 18
# speedup vs baseline: 1.0091x; 1.0091x over previous
"""Distributed multi-head attention (RoPE, non-causal) on 8 TRN2 NeuronCores.

Sharding: tensor-parallel over heads. Core c owns heads {2c, 2c+1}:
  - wq/wk/wv rows c*256:(c+1)*256 (output dim), x replicated (pre-transposed),
  - attention computed locally per (batch, head),
  - per-batch AllGather of the attention outputs (transposed layout, bf16),
  - each core then computes output columns c*256:(c+1)*256 with its wo rows.
Host side only shards/casts inputs and concatenates the 8 output column
slices -- all FLOPs run on device.

Structure: software-pipelined per batch so the ScalarE exp stream, the
AllGather, and the wo stage all overlap TensorE work of neighboring
batches:  proj(b) -> attn(b) -> AG(b) -> [wo(b-1)] ...

Layout/precision tricks:
  - All matmuls in bf16 (PSUM accumulates f32); rel-err ~5e-3.
  - RoPE pairs (even/odd head-dim) are separated into halves by permuting the
    wq/wk ROWS on the host, so on device RoPE is 4 full-width DVE ops against
    duplicated cos/sin tables. Permutation is applied identically to q and k,
    so q.k is unchanged.
  - Scores are computed transposed ([k, q]) so exp(scores) feeds attn@v as
    lhsT with no transpose; softmax denominator comes from a ones-column
    appended to v (matmul N=129). No max-subtraction: |scores| < ~10 here.
  - attn output normalized per-partition (q) then PE-transposed to [hd, q]
    so the AllGather concatenates cores along the o dim.
  - wo stage computes out.T (N=512 moving dim); host transposes back.
"""

import numpy as np
import ml_dtypes

B, S, D, H = 4, 2048, 2048, 16
HD = 128            # head dim
NCORES = 8
HPC = H // NCORES   # heads per core = 2
OSL = HPC * HD      # per-core o-slice = 256
ROWS = B * S        # 8192 flattened rows
DCH = D // 128      # 16 contraction chunks
SCH = 512           # seq chunk for projections
KB = S // 128       # 16 k-blocks per batch
QC = 512            # q chunk in attention
NQC = S // QC       # 4

BF16 = ml_dtypes.bfloat16
_NC_CACHE = None


def _build():
    import concourse.bass as bass  # noqa: F401
    import concourse.mybir as mybir
    import concourse.tile as tile
    from concourse import bacc
    from concourse.masks import make_identity

    fp32 = mybir.dt.float32
    bf16 = mybir.dt.bfloat16

    nc = bacc.Bacc(
        "TRN2",
        target_bir_lowering=False,
        debug=False,
        num_devices=NCORES,
    )

    xT = nc.declare_dram_parameter("xT", [D, ROWS], bf16, isOutput=False)
    wqT = nc.declare_dram_parameter("wqT", [D, OSL], bf16, isOutput=False)
    wkT = nc.declare_dram_parameter("wkT", [D, OSL], bf16, isOutput=False)
    wvT = nc.declare_dram_parameter("wvT", [D, OSL], bf16, isOutput=False)
    woT = nc.declare_dram_parameter("woT", [D, OSL], bf16, isOutput=False)
    cosd = nc.declare_dram_parameter("cosd", [128, S], fp32, isOutput=False)
    sind = nc.declare_dram_parameter("sind", [128, S], fp32, isOutput=False)
    outp = nc.declare_dram_parameter("out", [OSL, ROWS], fp32, isOutput=True)

    inv_sqrt_hd = 1.0 / float(np.sqrt(HD))

    with tile.TileContext(nc) as tc:
        with (
            tc.tile_pool(name="glob", bufs=1) as glob,
            tc.tile_pool(name="dram", bufs=1, space="DRAM") as dram,
            tc.tile_pool(name="qkv", bufs=2) as qkv,
            tc.tile_pool(name="xtp", bufs=3) as xtp,
            tc.tile_pool(name="attp", bufs=4) as attp,
            tc.tile_pool(name="gtp", bufs=3) as gtp,
            tc.tile_pool(name="tmpp", bufs=3) as tmpp,
            tc.tile_pool(name="smalls", bufs=4) as smalls,
            tc.tile_pool(name="otp", bufs=2) as otp,
            tc.tile_pool(name="psA", bufs=3, space="PSUM") as psA,
            tc.tile_pool(name="psB", bufs=3, space="PSUM") as psB,
            tc.tile_pool(name="psCD", bufs=2, space="PSUM") as psCD,
        ):
            ident = glob.tile([128, 128], bf16, name="ident")
            make_identity(nc, ident[:])

            wq_sb = glob.tile([128, DCH, OSL], bf16, name="wq_sb")
            wk_sb = glob.tile([128, DCH, OSL], bf16, name="wk_sb")
            wv_sb = glob.tile([128, DCH, OSL], bf16, name="wv_sb")
            wo_sb = glob.tile([128, DCH, OSL], bf16, name="wo_sb")
            cosb = glob.tile([128, S], fp32, name="cosb")
            sinb = glob.tile([128, S], fp32, name="sinb")
            # startup order: what the first projection needs, first
            nc.gpsimd.dma_start(
                wq_sb[:], wqT[:, :].rearrange("(c p) n -> p c n", p=128))
            xh00 = []
            for half in range(2):
                xth = xtp.tile([128, DCH // 2, SCH], bf16,
                               name=f"xt0_{half}", tag="xt")
                nc.gpsimd.dma_start(
                    xth[:],
                    xT[half * 1024:(half + 1) * 1024, 0:SCH]
                    .rearrange("(c p) n -> p c n", p=128))
                xh00.append(xth)
            nc.gpsimd.dma_start(cosb[:], cosd[:, :])
            nc.gpsimd.dma_start(sinb[:], sind[:, :])
            nc.gpsimd.dma_start(
                wk_sb[:], wkT[:, :].rearrange("(c p) n -> p c n", p=128))
            nc.gpsimd.dma_start(
                wv_sb[:], wvT[:, :].rearrange("(c p) n -> p c n", p=128))

            # half-batch granularity: cols [0:1024) and [1024:2048) of each
            # batch gather/project independently to shrink the serial tail
            HB = S // 2
            bounce = [dram.tile([OSL, HB], bf16, name=f"bounce{u}")
                      for u in range(2 * B)]
            gath = [dram.tile([NCORES * OSL, HB], bf16, addr_space="Shared",
                              name=f"gath{u}") for u in range(2 * B)]

            def fetch_x(b, sc):
                col0 = b * S + sc * SCH
                xh = []
                for half in range(2):
                    xth = xtp.tile([128, DCH // 2, SCH], bf16,
                                   name=f"xt{half}", tag="xt")
                    nc.gpsimd.dma_start(
                        xth[:],
                        xT[half * 1024:(half + 1) * 1024, col0:col0 + SCH]
                        .rearrange("(c p) n -> p c n", p=128))
                    xh.append(xth)
                return xh

            def proj(b, xh_pre):
                qt = qkv.tile([128, HPC, S], bf16, name="qt", tag="qt")
                kt = qkv.tile([128, HPC, S], bf16, name="kt", tag="kt")
                vt = qkv.tile([128, KB, HPC, HD + 1], bf16, name="vt",
                              tag="vt")
                nc.vector.memset(vt[:, :, :, HD:HD + 1], 1.0)
                for sc in range(S // SCH):
                    xh = xh_pre if sc == 0 else fetch_x(b, sc)
                    cosr = cosb[:, sc * SCH:(sc + 1) * SCH]
                    sinr = sinb[:, sc * SCH:(sc + 1) * SCH]
                    for (w_sb, dstT) in ((wq_sb, qt), (wk_sb, kt)):
                        for h in range(HPC):
                            ps = psA.tile([128, SCH], fp32, name="ps_proj",
                                          tag="psA")
                            for c in range(DCH):
                                nc.tensor.matmul(
                                    ps[:],
                                    w_sb[:, c, h * HD:(h + 1) * HD],
                                    xh[c // 8][:, c % 8, :],
                                    start=(c == 0), stop=(c == DCH - 1))
                            m1 = tmpp.tile([128, SCH], fp32, name="m1",
                                           tag="t")
                            m2 = tmpp.tile([128, SCH], fp32, name="m2",
                                           tag="t")
                            # m1 = [tr*cos ; ti*cos]; m2 swapped-halves =
                            # [ti*sin ; tr*sin] so later DVE ops use equal
                            # SBUF base partitions (PSUM operand may differ).
                            nc.vector.tensor_mul(m1[:], ps[:], cosr)
                            nc.vector.tensor_mul(
                                m2[0:64, :], ps[64:128, :], sinr[0:64, :])
                            nc.vector.tensor_mul(
                                m2[64:128, :], ps[0:64, :], sinr[64:128, :])
                            sl = slice(sc * SCH, (sc + 1) * SCH)
                            nc.vector.tensor_sub(
                                dstT[0:64, h, sl], m1[0:64, :], m2[0:64, :])
                            nc.vector.tensor_add(
                                dstT[64:128, h, sl], m2[64:128, :],
                                m1[64:128, :])
                    for ssb in range(SCH // 128):
                        kb = sc * (SCH // 128) + ssb
                        psv = psA.tile([128, OSL], fp32, name="psv", tag="psA")
                        for c in range(DCH):
                            nc.tensor.matmul(
                                psv[:],
                                xh[c // 8][:, c % 8, ssb * 128:(ssb + 1) * 128],
                                wv_sb[:, c, :],
                                start=(c == 0), stop=(c == DCH - 1))
                        nc.vector.tensor_copy(
                            vt[:, kb, :, 0:HD],
                            psv[:].rearrange("p (h d) -> p h d", h=HPC))
                return qt, kt, vt

            def attn_scores(qt, kt, h, qc):
                halves = []
                for eh in range(2):
                    expT = attp.tile([128, KB // 2, QC], bf16, name="expT",
                                     tag="expT")
                    for j in range(KB // 2):
                        kb = eh * (KB // 2) + j
                        pss = psB.tile([128, QC], fp32, name="pss", tag="psB")
                        nc.tensor.matmul(
                            pss[:],
                            kt[:, h, kb * 128:(kb + 1) * 128],
                            qt[:, h, qc * QC:(qc + 1) * QC],
                            start=True, stop=True)
                        nc.scalar.activation(
                            expT[:, j, :], pss[:],
                            mybir.ActivationFunctionType.Exp,
                            scale=inv_sqrt_hd)
                    halves.append(expT)
                return halves

            def attn_v(vt, expT, b, h, qc):
                u = 2 * b + qc // 2
                for qsb in range(QC // 128):
                    pso = psCD.tile([128, HD + 1], fp32, name="pso",
                                    tag="psCD")
                    for kb in range(KB):
                        nc.tensor.matmul(
                            pso[:],
                            expT[kb // (KB // 2)][:, kb % (KB // 2),
                                                  qsb * 128:(qsb + 1) * 128],
                            vt[:, kb, h, :],
                            start=(kb == 0), stop=(kb == KB - 1))
                    rc = smalls.tile([128, 1], fp32, name="rc", tag="rc")
                    nc.vector.reciprocal(rc[:], pso[:, HD:HD + 1])
                    a_sb = smalls.tile([128, HD], bf16, name="a_sb",
                                       tag="a_sb")
                    nc.vector.tensor_scalar_mul(a_sb[:], pso[:, 0:HD], rc[:])
                    pst = psCD.tile([128, 128], bf16, name="pst", tag="psCD")
                    nc.tensor.transpose(pst[:], a_sb[:], ident[:])
                    a_t = smalls.tile([128, 128], bf16, name="a_t", tag="a_t")
                    nc.vector.tensor_copy(a_t[:], pst[:])
                    col0 = (qc % 2) * QC + qsb * 128
                    nc.gpsimd.dma_start(
                        bounce[u][h * HD:(h + 1) * HD, col0:col0 + 128],
                        a_t[:])

            def attention(b, qt, kt, vt, post_half):
                units = [(h, qc) for qc in range(NQC) for h in range(HPC)]
                pend = []
                done = [0]

                def flush_one():
                    eT, ph, pqc = pend.pop(0)
                    attn_v(vt, eT, b, ph, pqc)
                    done[0] += 1
                    if done[0] == 4:
                        post_half(0)
                    elif done[0] == 8:
                        post_half(1)

                for (h, qc) in units:
                    expT = attn_scores(qt, kt, h, qc)
                    pend.append((expT, h, qc))
                    if len(pend) > 1:
                        flush_one()
                flush_one()

            def allgather(u):
                nc.gpsimd.collective_compute(
                    "AllGather",
                    mybir.AluOpType.bypass,
                    ins=[bounce[u].opt()],
                    outs=[gath[u].opt()],
                    replica_groups=[list(range(NCORES))],
                )

            def wo_stage(b, half):
                u = 2 * b + half
                for rc_ in range(2):
                    gh = []
                    for dh in range(2):
                        g = gtp.tile([128, DCH // 2, 512], bf16,
                                     name=f"gt{dh}", tag="gt")
                        nc.scalar.dma_start(
                            g[:],
                            gath[u][dh * 1024:(dh + 1) * 1024,
                                    rc_ * 512:(rc_ + 1) * 512]
                            .rearrange("(c p) n -> p c n", p=128))
                        gh.append(g)
                    for oc in range(OSL // 128):
                        psw = psA.tile([128, 512], fp32, name="psw", tag="psA")
                        for c in range(DCH):
                            nc.tensor.matmul(
                                psw[:],
                                wo_sb[:, c, oc * 128:(oc + 1) * 128],
                                gh[c // 8][:, c % 8, :],
                                start=(c == 0), stop=(c == DCH - 1))
                        out_t = otp.tile([128, 512], fp32, name="out_t",
                                         tag="out_t")
                        nc.vector.tensor_copy(out_t[:], psw[:])
                        col0 = b * S + half * HB + rc_ * 512
                        nc.scalar.dma_start(
                            outp[oc * 128:(oc + 1) * 128, col0:col0 + 512],
                            out_t[:])

            wo_loaded = [False]

            def make_post(b):
                def cb(half):
                    allgather(2 * b + half)
                    if not wo_loaded[0]:
                        nc.gpsimd.dma_start(
                            wo_sb[:],
                            woT[:, :].rearrange("(c p) n -> p c n", p=128))
                        wo_loaded[0] = True
                    if b >= 1:
                        wo_stage(b - 1, half)
                return cb

            xh_pre = xh00
            for b in range(B):
                qt, kt, vt = proj(b, xh_pre)
                if b + 1 < B:
                    xh_pre = fetch_x(b + 1, 0)
                attention(b, qt, kt, vt, make_post(b))
            wo_stage(B - 1, 0)
            wo_stage(B - 1, 1)

    nc.compile()
    return nc


def _shard_inputs(x, freqs_cos, freqs_sin, wq, wk, wv, wo):
    xf = np.asarray(x, dtype=np.float32).reshape(ROWS, D)
    xT = np.ascontiguousarray(xf.T).astype(BF16)
    fcT = np.asarray(freqs_cos, dtype=np.float32).T  # [64, S]
    fsT = np.asarray(freqs_sin, dtype=np.float32).T
    cosd = np.ascontiguousarray(np.concatenate([fcT, fcT], 0))  # [128, S]
    sind = np.ascontiguousarray(np.concatenate([fsT, fsT], 0))
    # even indices (real half) then odd (imag half), per head
    perm = np.concatenate([np.arange(0, HD, 2), np.arange(1, HD, 2)])
    in_maps = []
    for c in range(NCORES):
        rows = slice(c * OSL, (c + 1) * OSL)
        wq_c = np.asarray(wq)[rows].reshape(HPC, HD, D)[:, perm, :].reshape(OSL, D)
        wk_c = np.asarray(wk)[rows].reshape(HPC, HD, D)[:, perm, :].reshape(OSL, D)
        in_maps.append({
            "xT": xT,
            "wqT": np.ascontiguousarray(wq_c.T).astype(BF16),
            "wkT": np.ascontiguousarray(wk_c.T).astype(BF16),
            "wvT": np.ascontiguousarray(np.asarray(wv)[rows].T).astype(BF16),
            "woT": np.ascontiguousarray(np.asarray(wo)[rows].T).astype(BF16),
            "cosd": cosd,
            "sind": sind,
        })
    return in_maps


def run(inputs, trace=False, trace_cores=None):
    """Build (cached), run on 8 cores; returns (full_output, BassKernelResults)."""
    global _NC_CACHE
    from concourse.bass_utils import run_bass_kernel_spmd
    if _NC_CACHE is None:
        _NC_CACHE = _build()
    in_maps = _shard_inputs(**inputs)
    res = run_bass_kernel_spmd(
        _NC_CACHE, in_maps, core_ids=list(range(NCORES)), trace=trace,
        trace_cores=trace_cores)
    parts = [np.ascontiguousarray(
        np.asarray(res.results[c]["out"], dtype=np.float32).T)
        for c in range(NCORES)]
    full = np.concatenate(parts, axis=1).reshape(B, S, D)
    return full, res


def kernel(x, freqs_cos, freqs_sin, wq, wk, wv, wo):
    full, _ = run(dict(x=x, freqs_cos=freqs_cos, freqs_sin=freqs_sin,
                       wq=wq, wk=wk, wv=wv, wo=wo))
    return full


# revision 19
# speedup vs baseline: 1.0482x; 1.0387x over previous
"""Distributed multi-head attention (RoPE, non-causal) on 8 TRN2 NeuronCores.

Sharding: tensor-parallel over heads. Core c owns heads {2c, 2c+1}:
  - wq/wk/wv rows c*256:(c+1)*256 (output dim), x replicated (pre-transposed),
  - attention computed locally per (batch, head),
  - per-batch AllGather of the attention outputs (transposed layout, bf16),
  - each core then computes output columns c*256:(c+1)*256 with its wo rows.
Host side only shards/casts inputs and concatenates the 8 output column
slices -- all FLOPs run on device.

Structure: software-pipelined per batch so the ScalarE exp stream, the
AllGather, and the wo stage all overlap TensorE work of neighboring
batches:  proj(b) -> attn(b) -> AG(b) -> [wo(b-1)] ...

Layout/precision tricks:
  - All matmuls in bf16 (PSUM accumulates f32); rel-err ~5e-3.
  - RoPE pairs (even/odd head-dim) are separated into halves by permuting the
    wq/wk ROWS on the host, so on device RoPE is 4 full-width DVE ops against
    duplicated cos/sin tables. Permutation is applied identically to q and k,
    so q.k is unchanged.
  - Scores are computed transposed ([k, q]) so exp(scores) feeds attn@v as
    lhsT with no transpose; softmax denominator comes from a ones-column
    appended to v (matmul N=129). No max-subtraction: |scores| < ~10 here.
  - attn output normalized per-partition (q) then PE-transposed to [hd, q]
    so the AllGather concatenates cores along the o dim.
  - wo stage computes out.T (N=512 moving dim); host transposes back.
"""

import numpy as np
import ml_dtypes

B, S, D, H = 4, 2048, 2048, 16
HD = 128            # head dim
NCORES = 8
HPC = H // NCORES   # heads per core = 2
OSL = HPC * HD      # per-core o-slice = 256
ROWS = B * S        # 8192 flattened rows
DCH = D // 128      # 16 contraction chunks
SCH = 512           # seq chunk for projections
KB = S // 128       # 16 k-blocks per batch
QC = 512            # q chunk in attention
NQC = S // QC       # 4

BF16 = ml_dtypes.bfloat16
_NC_CACHE = None


def _build():
    import concourse.bass as bass  # noqa: F401
    import concourse.mybir as mybir
    import concourse.tile as tile
    from concourse import bacc
    from concourse.masks import make_identity

    fp32 = mybir.dt.float32
    bf16 = mybir.dt.bfloat16

    nc = bacc.Bacc(
        "TRN2",
        target_bir_lowering=False,
        debug=False,
        num_devices=NCORES,
    )

    xT = nc.declare_dram_parameter("xT", [D, ROWS], bf16, isOutput=False)
    wqT = nc.declare_dram_parameter("wqT", [D, OSL], bf16, isOutput=False)
    wkT = nc.declare_dram_parameter("wkT", [D, OSL], bf16, isOutput=False)
    wvT = nc.declare_dram_parameter("wvT", [D, OSL], bf16, isOutput=False)
    woT = nc.declare_dram_parameter("woT", [D, OSL], bf16, isOutput=False)
    cosd = nc.declare_dram_parameter("cosd", [128, S], fp32, isOutput=False)
    sind = nc.declare_dram_parameter("sind", [128, S], fp32, isOutput=False)
    outp = nc.declare_dram_parameter("out", [OSL, ROWS], fp32, isOutput=True)

    inv_sqrt_hd = 1.0 / float(np.sqrt(HD))

    with tile.TileContext(nc) as tc:
        with (
            tc.tile_pool(name="glob", bufs=1) as glob,
            tc.tile_pool(name="dram", bufs=1, space="DRAM") as dram,
            tc.tile_pool(name="qkv", bufs=2) as qkv,
            tc.tile_pool(name="xtp", bufs=3) as xtp,
            tc.tile_pool(name="attp", bufs=4) as attp,
            tc.tile_pool(name="gtp", bufs=3) as gtp,
            tc.tile_pool(name="tmpp", bufs=3) as tmpp,
            tc.tile_pool(name="smalls", bufs=4) as smalls,
            tc.tile_pool(name="otp", bufs=2) as otp,
            tc.tile_pool(name="psA", bufs=3, space="PSUM") as psA,
            tc.tile_pool(name="psB", bufs=3, space="PSUM") as psB,
            tc.tile_pool(name="psCD", bufs=2, space="PSUM") as psCD,
        ):
            ident = glob.tile([128, 128], bf16, name="ident")
            make_identity(nc, ident[:])

            wq_sb = glob.tile([128, DCH, OSL], bf16, name="wq_sb")
            wk_sb = glob.tile([128, DCH, OSL], bf16, name="wk_sb")
            wv_sb = glob.tile([128, DCH, OSL], bf16, name="wv_sb")
            wo_sb = glob.tile([128, DCH, OSL], bf16, name="wo_sb")
            cosb = glob.tile([128, S], fp32, name="cosb")
            sinb = glob.tile([128, S], fp32, name="sinb")
            # startup order: what the first projection needs, first
            nc.gpsimd.dma_start(
                wq_sb[:], wqT[:, :].rearrange("(c p) n -> p c n", p=128))
            xh00 = []
            for half in range(2):
                xth = xtp.tile([128, DCH // 2, SCH], bf16,
                               name=f"xt0_{half}", tag="xt")
                nc.gpsimd.dma_start(
                    xth[:],
                    xT[half * 1024:(half + 1) * 1024, 0:SCH]
                    .rearrange("(c p) n -> p c n", p=128))
                xh00.append(xth)
            nc.gpsimd.dma_start(cosb[:], cosd[:, :])
            nc.gpsimd.dma_start(sinb[:], sind[:, :])
            nc.gpsimd.dma_start(
                wk_sb[:], wkT[:, :].rearrange("(c p) n -> p c n", p=128))
            nc.gpsimd.dma_start(
                wv_sb[:], wvT[:, :].rearrange("(c p) n -> p c n", p=128))

            # half-batch granularity: cols [0:1024) and [1024:2048) of each
            # batch gather/project independently to shrink the serial tail
            HB = S // 2
            bounce = [dram.tile([OSL, HB], bf16, name=f"bounce{u}")
                      for u in range(2 * B)]
            gath = [dram.tile([NCORES * OSL, HB], bf16, addr_space="Shared",
                              name=f"gath{u}") for u in range(2 * B)]

            def fetch_x(b, sc):
                col0 = b * S + sc * SCH
                xh = []
                for half in range(2):
                    xth = xtp.tile([128, DCH // 2, SCH], bf16,
                                   name=f"xt{half}", tag="xt")
                    nc.gpsimd.dma_start(
                        xth[:],
                        xT[half * 1024:(half + 1) * 1024, col0:col0 + SCH]
                        .rearrange("(c p) n -> p c n", p=128))
                    xh.append(xth)
                return xh

            def proj(b, xh_pre):
                qt = qkv.tile([128, HPC, S], bf16, name="qt", tag="qt")
                kt = qkv.tile([128, HPC, S], bf16, name="kt", tag="kt")
                vt = qkv.tile([128, KB, HPC, HD + 1], bf16, name="vt",
                              tag="vt")
                nc.vector.memset(vt[:, :, :, HD:HD + 1], 1.0)
                for sc in range(S // SCH):
                    xh = xh_pre if sc == 0 else fetch_x(b, sc)
                    cosr = cosb[:, sc * SCH:(sc + 1) * SCH]
                    sinr = sinb[:, sc * SCH:(sc + 1) * SCH]
                    for (w_sb, dstT) in ((wq_sb, qt), (wk_sb, kt)):
                        for h in range(HPC):
                            ps = psA.tile([128, SCH], fp32, name="ps_proj",
                                          tag="psA")
                            for c in range(DCH):
                                nc.tensor.matmul(
                                    ps[:],
                                    w_sb[:, c, h * HD:(h + 1) * HD],
                                    xh[c // 8][:, c % 8, :],
                                    start=(c == 0), stop=(c == DCH - 1))
                            m1 = tmpp.tile([128, SCH], fp32, name="m1",
                                           tag="t")
                            m2 = tmpp.tile([128, SCH], fp32, name="m2",
                                           tag="t")
                            # m1 = [tr*cos ; ti*cos]; m2 swapped-halves =
                            # [ti*sin ; tr*sin] so later DVE ops use equal
                            # SBUF base partitions (PSUM operand may differ).
                            nc.vector.tensor_mul(m1[:], ps[:], cosr)
                            nc.vector.tensor_mul(
                                m2[0:64, :], ps[64:128, :], sinr[0:64, :])
                            nc.vector.tensor_mul(
                                m2[64:128, :], ps[0:64, :], sinr[64:128, :])
                            sl = slice(sc * SCH, (sc + 1) * SCH)
                            nc.vector.tensor_sub(
                                dstT[0:64, h, sl], m1[0:64, :], m2[0:64, :])
                            nc.vector.tensor_add(
                                dstT[64:128, h, sl], m2[64:128, :],
                                m1[64:128, :])
                    for ssb in range(SCH // 128):
                        kb = sc * (SCH // 128) + ssb
                        psv = psA.tile([128, OSL], fp32, name="psv", tag="psA")
                        for c in range(DCH):
                            nc.tensor.matmul(
                                psv[:],
                                xh[c // 8][:, c % 8, ssb * 128:(ssb + 1) * 128],
                                wv_sb[:, c, :],
                                start=(c == 0), stop=(c == DCH - 1))
                        nc.vector.tensor_copy(
                            vt[:, kb, :, 0:HD],
                            psv[:].rearrange("p (h d) -> p h d", h=HPC))
                return qt, kt, vt

            def attn_scores(qt, kt, h, qc):
                halves = []
                for eh in range(2):
                    expT = attp.tile([128, KB // 2, QC], bf16, name="expT",
                                     tag="expT")
                    for j in range(KB // 2):
                        kb = eh * (KB // 2) + j
                        pss = psB.tile([128, QC], fp32, name="pss", tag="psB")
                        nc.tensor.matmul(
                            pss[:],
                            kt[:, h, kb * 128:(kb + 1) * 128],
                            qt[:, h, qc * QC:(qc + 1) * QC],
                            start=True, stop=True)
                        nc.scalar.activation(
                            expT[:, j, :], pss[:],
                            mybir.ActivationFunctionType.Exp,
                            scale=inv_sqrt_hd)
                    halves.append(expT)
                return halves

            def attn_v(vt, expT, b, h, qc):
                u = 2 * b + qc // 2
                a_t = smalls.tile([128, QC], bf16, name="a_t", tag="a_t")
                for qsb in range(QC // 128):
                    pso = psCD.tile([128, HD + 1], fp32, name="pso",
                                    tag="psCD")
                    for kb in range(KB):
                        nc.tensor.matmul(
                            pso[:],
                            expT[kb // (KB // 2)][:, kb % (KB // 2),
                                                  qsb * 128:(qsb + 1) * 128],
                            vt[:, kb, h, :],
                            start=(kb == 0), stop=(kb == KB - 1))
                    rc = smalls.tile([128, 1], fp32, name="rc", tag="rc")
                    nc.vector.reciprocal(rc[:], pso[:, HD:HD + 1])
                    a_sb = smalls.tile([128, HD], bf16, name="a_sb",
                                       tag="a_sb")
                    nc.vector.tensor_scalar_mul(a_sb[:], pso[:, 0:HD], rc[:])
                    pst = psCD.tile([128, 128], bf16, name="pst", tag="psCD")
                    nc.tensor.transpose(pst[:], a_sb[:], ident[:])
                    nc.vector.tensor_copy(
                        a_t[:, qsb * 128:(qsb + 1) * 128], pst[:])
                col0 = (qc % 2) * QC
                nc.gpsimd.dma_start(
                    bounce[u][h * HD:(h + 1) * HD, col0:col0 + QC], a_t[:])

            def attention(b, qt, kt, vt, post_half):
                units = [(h, qc) for qc in range(NQC) for h in range(HPC)]
                pend = []
                done = [0]

                def flush_one():
                    eT, ph, pqc = pend.pop(0)
                    attn_v(vt, eT, b, ph, pqc)
                    done[0] += 1
                    if done[0] == 4:
                        post_half(0)
                    elif done[0] == 8:
                        post_half(1)

                for (h, qc) in units:
                    expT = attn_scores(qt, kt, h, qc)
                    pend.append((expT, h, qc))
                    if len(pend) > 1:
                        flush_one()
                flush_one()

            def allgather(u):
                nc.gpsimd.collective_compute(
                    "AllGather",
                    mybir.AluOpType.bypass,
                    ins=[bounce[u].opt()],
                    outs=[gath[u].opt()],
                    replica_groups=[list(range(NCORES))],
                )

            def wo_stage(b, half):
                u = 2 * b + half
                for rc_ in range(2):
                    gh = []
                    for dh in range(2):
                        g = gtp.tile([128, DCH // 2, 512], bf16,
                                     name=f"gt{dh}", tag="gt")
                        nc.scalar.dma_start(
                            g[:],
                            gath[u][dh * 1024:(dh + 1) * 1024,
                                    rc_ * 512:(rc_ + 1) * 512]
                            .rearrange("(c p) n -> p c n", p=128))
                        gh.append(g)
                    for oc in range(OSL // 128):
                        psw = psA.tile([128, 512], fp32, name="psw", tag="psA")
                        for c in range(DCH):
                            nc.tensor.matmul(
                                psw[:],
                                wo_sb[:, c, oc * 128:(oc + 1) * 128],
                                gh[c // 8][:, c % 8, :],
                                start=(c == 0), stop=(c == DCH - 1))
                        out_t = otp.tile([128, 512], fp32, name="out_t",
                                         tag="out_t")
                        nc.vector.tensor_copy(out_t[:], psw[:])
                        col0 = b * S + half * HB + rc_ * 512
                        nc.scalar.dma_start(
                            outp[oc * 128:(oc + 1) * 128, col0:col0 + 512],
                            out_t[:])

            wo_loaded = [False]

            def make_post(b):
                def cb(half):
                    allgather(2 * b + half)
                    if not wo_loaded[0]:
                        nc.gpsimd.dma_start(
                            wo_sb[:],
                            woT[:, :].rearrange("(c p) n -> p c n", p=128))
                        wo_loaded[0] = True
                    if b >= 1:
                        wo_stage(b - 1, half)
                return cb

            xh_pre = xh00
            for b in range(B):
                qt, kt, vt = proj(b, xh_pre)
                if b + 1 < B:
                    xh_pre = fetch_x(b + 1, 0)
                attention(b, qt, kt, vt, make_post(b))
            wo_stage(B - 1, 0)
            wo_stage(B - 1, 1)

    nc.compile()
    return nc


def _shard_inputs(x, freqs_cos, freqs_sin, wq, wk, wv, wo):
    xf = np.asarray(x, dtype=np.float32).reshape(ROWS, D)
    xT = np.ascontiguousarray(xf.T).astype(BF16)
    fcT = np.asarray(freqs_cos, dtype=np.float32).T  # [64, S]
    fsT = np.asarray(freqs_sin, dtype=np.float32).T
    cosd = np.ascontiguousarray(np.concatenate([fcT, fcT], 0))  # [128, S]
    sind = np.ascontiguousarray(np.concatenate([fsT, fsT], 0))
    # even indices (real half) then odd (imag half), per head
    perm = np.concatenate([np.arange(0, HD, 2), np.arange(1, HD, 2)])
    in_maps = []
    for c in range(NCORES):
        rows = slice(c * OSL, (c + 1) * OSL)
        wq_c = np.asarray(wq)[rows].reshape(HPC, HD, D)[:, perm, :].reshape(OSL, D)
        wk_c = np.asarray(wk)[rows].reshape(HPC, HD, D)[:, perm, :].reshape(OSL, D)
        in_maps.append({
            "xT": xT,
            "wqT": np.ascontiguousarray(wq_c.T).astype(BF16),
            "wkT": np.ascontiguousarray(wk_c.T).astype(BF16),
            "wvT": np.ascontiguousarray(np.asarray(wv)[rows].T).astype(BF16),
            "woT": np.ascontiguousarray(np.asarray(wo)[rows].T).astype(BF16),
            "cosd": cosd,
            "sind": sind,
        })
    return in_maps


def run(inputs, trace=False, trace_cores=None):
    """Build (cached), run on 8 cores; returns (full_output, BassKernelResults)."""
    global _NC_CACHE
    from concourse.bass_utils import run_bass_kernel_spmd
    if _NC_CACHE is None:
        _NC_CACHE = _build()
    in_maps = _shard_inputs(**inputs)
    res = run_bass_kernel_spmd(
        _NC_CACHE, in_maps, core_ids=list(range(NCORES)), trace=trace,
        trace_cores=trace_cores)
    parts = [np.ascontiguousarray(
        np.asarray(res.results[c]["out"], dtype=np.float32).T)
        for c in range(NCORES)]
    full = np.concatenate(parts, axis=1).reshape(B, S, D)
    return full, res


def kernel(x, freqs_cos, freqs_sin, wq, wk, wv, wo):
    full, _ = run(dict(x=x, freqs_cos=freqs_cos, freqs_sin=freqs_sin,
                       wq=wq, wk=wk, wv=wv, wo=wo))
    return full
